# revision 11
# baseline (speedup 1.0000x reference)
"""Trainium2 Bass kernel for KAN Fourier linear layer.

y[b, j] = sum_{i,k} cos(k x[b,i]) W0[j,i,k] + sin(k x[b,i]) W1[j,i,k] + bias[j]

Strategy (8 cores, data-parallel over batch; fp16 matmul; trig generated
on-device with the work spread over DVE, ACT and GpSimd so it all hides
under the PE's ~218us of fp16 matmul):

  - Each core: B=1024 batch rows, all of (i, k, j); W replicated (fp16).
  - Trig tiles are [128, 2048] fp16: partition p = i%128, free dim =
    (i//128)*1024 + b.  PE contracts 128 i-rows per matmul; 64k x 2trig x
    2ih x 2jh x 2bh = 1024 matmuls of [128,128]x[128,512].
  - Per-k generation DAG:
      direct  (26 k): magic-round range reduction (u,v on DVE; f=u-v on
                GpSimd), |f| + sin + cos on ACT (all in one act table).
      double  (27 k): s2=2*s*c, c2=1-2s^2 from parent k/2 (sq,c2,s2 DVE;
                s*c product on GpSimd).
      triple  (11 k): s3=s(3-4s^2), c3=c(1-4s^2) from parent k/3
                (square on ACT; the rest DVE).
    fp16 carry keeps chain error ~2^-11 per hop; worst chain depth 2.
"""

import numpy as np

import concourse.bacc as bacc
import concourse.mybir as mybir
import concourse.tile as tile
from concourse import bass_utils

N_CORES = 8
B_FULL = 8192
B = B_FULL // N_CORES  # 1024 batch rows per core
I = 256
K = 64
J = 256
P = 128
F = 2048  # free dim of trig tiles: (i_half, b)

# Per-k generation plan.
TRIPLE = {3: 1, 9: 3, 15: 5, 21: 7, 33: 11, 39: 13, 45: 15, 51: 17,
          54: 18, 57: 19, 63: 21}
DIRECT = {1, 5, 7, 11, 12, 13, 16, 17, 18, 19, 23, 25, 27, 29, 31, 35,
          37, 41, 43, 47, 49, 53, 55, 59, 60, 61}
DOUBLE = {k: k // 2 for k in range(2, K + 1, 2) if k not in DIRECT and k not in TRIPLE}

# Approximate engine cost (us) per k by class, used to interleave the
# emission order so DVE/ACT/Pool cumulative load stays balanced (the tile
# scheduler follows emission order closely; a run of same-class ks leaves
# two engines idle).
_COST = {
    "direct": (2.44, 6.33, 2.94),   # (DVE, ACT, POOL)
    "double": (2.45, 0.0, 4.16),
    "triple": (3.79, 2.11, 0.0),
}


def _class_of(k):
    return "direct" if k in DIRECT else ("double" if k in DOUBLE else "triple")


def _make_order():
    # directs ordered so chain parents come early
    d_parents = [1, 5, 7, 11, 13, 12, 16, 17, 18, 19, 23, 25, 29, 31]
    head = [1, 5, 2, 7, 3, 4]
    d_rest = [27, 60, 35, 37, 41, 43, 47, 49, 53, 55, 59, 61]
    d_stream = d_parents + d_rest
    emitted = set(head)
    order = list(head)
    load = [0.0, 0.0, 0.0]
    for k in head:
        c = _COST[_class_of(k)]
        load = [load[i] + c[i] for i in range(3)]
    d_stream = [k for k in d_stream if k not in emitted]
    di = 0
    chain_ready = []

    def refresh_ready():
        for k in range(2, K + 1):
            if k in emitted or k in chain_ready or k in DIRECT:
                continue
            par = TRIPLE.get(k) or DOUBLE.get(k)
            if par in emitted:
                chain_ready.append(k)

    while len(order) < K:
        refresh_ready()
        cands = []
        if di < len(d_stream):
            cands.append(("direct", d_stream[di]))
        if chain_ready:
            cands.append((_class_of(chain_ready[0]), chain_ready[0]))
        # pick the candidate minimizing the resulting max engine load
        best = None
        for cls, k in cands:
            c = _COST[cls]
            new = [load[i] + c[i] for i in range(3)]
            key = max(new)
            if best is None or key < best[0]:
                best = (key, cls, k, new)
        _, cls, k, new = best
        load = new
        if cls == "direct" and di < len(d_stream) and d_stream[di] == k:
            di += 1
        else:
            chain_ready.remove(k)
        emitted.add(k)
        order.append(k)
    # keep the tail cheap: swap trailing directs earlier (directs have no
    # parents so moving them earlier is topologically safe; doubles/triples
    # moving later is also safe)
    for pos in range(K - 4, K):
        if order[pos] in DIRECT:
            for q in range(pos - 1, -1, -1):
                if order[q] not in DIRECT and order[q] not in _chain_parents():
                    order[pos], order[q] = order[q], order[pos]
                    break
    return order


def _chain_parents():
    return set(TRIPLE.values()) | set(DOUBLE.values())


ORDER = _make_order()
assert set(ORDER) == set(range(1, K + 1)) and len(ORDER) == K
for k, p_ in {**TRIPLE, **DOUBLE}.items():
    assert ORDER.index(p_) < ORDER.index(k)

TRIG_SLOTS = 14

_cache = {}


def _plan_slots():
    """Assign each k an (s,c) tile slot, register-allocator style."""
    idx = {k: i for i, k in enumerate(ORDER)}
    last_use = {k: min(i + 3, K - 1) for k, i in idx.items()}  # mm DELAY later
    for k in ORDER:
        p_ = TRIPLE.get(k) or DOUBLE.get(k)
        if p_ is not None:
            last_use[p_] = max(last_use[p_], idx[k])
    slot_of = {}
    free = list(range(TRIG_SLOTS))[::-1]
    active = []  # (last_use, k)
    for i, k in enumerate(ORDER):
        active = [(lu, kk) for lu, kk in active if lu >= i]
        used = {slot_of[kk] for _, kk in active}
        free = [s for s in range(TRIG_SLOTS) if s not in used][::-1]
        assert free, f"out of trig slots at k={k}"
        s = free.pop()
        slot_of[k] = s
        active.append((last_use[k], k))
    return slot_of


def _build():
    if "nc" in _cache:
        return _cache["nc"]

    f32 = mybir.dt.float32
    f16 = mybir.dt.float16
    u32 = mybir.dt.uint32
    nc = bacc.Bacc("TRN2", target_bir_lowering=False, debug=False, num_devices=N_CORES)

    xb_dram = nc.dram_tensor("xb", (P, F), f32, kind="ExternalInput")
    w_dram = nc.dram_tensor("w", (K, P, 1024), f16, kind="ExternalInput")
    bias_dram = nc.dram_tensor("bias", (1, J), f16, kind="ExternalInput")
    yT_dram = nc.dram_tensor("yT", (J, B), f32, kind="ExternalOutput")

    TWO_PI = float(2.0 * np.pi)
    PI = float(np.pi)
    MAGIC = float(1.5 * 2.0**23)
    Alu = mybir.AluOpType
    Act = mybir.ActivationFunctionType

    slot_of = _plan_slots()
    n_mm_per_acc = K * 4 + 1  # trig x ih per k, plus the bias matmul
    mm_count = {(jh, bh): 0 for jh in range(2) for bh in range(2)}

    with tile.TileContext(nc) as tc:
        with (
            tc.tile_pool(name="const", bufs=1) as const_pool,
            tc.tile_pool(name="wload", bufs=5) as w_pool,
            tc.tile_pool(name="red", bufs=2) as red_pool,
            tc.tile_pool(name="trig", bufs=1) as trig_pool,
            tc.tile_pool(name="psum", bufs=1, space="PSUM") as psum_pool,
            tc.tile_pool(name="out", bufs=2) as out_pool,
        ):
            xb = const_pool.tile([P, F], f32, tag="xb")
            # split the x DMA so the first seed can start sooner
            for q in range(4):
                nc.sync.dma_start(xb[:, q * 512:(q + 1) * 512],
                                  xb_dram[:, q * 512:(q + 1) * 512])

            bias_sb = const_pool.tile([1, J], f16, tag="bias")
            nc.sync.dma_start(bias_sb[:], bias_dram[:, :])
            ones_sb = const_pool.tile([1, 512], f16, tag="ones")
            nc.vector.memset(ones_sb[:], 1.0)
            pi_half = const_pool.tile([P, 1], f32, tag="pi_half")
            nc.vector.memset(pi_half[:], PI / 2)

            accs = [
                [psum_pool.tile([P, 512], f32, tag=f"acc{j}{b}", name=f"acc{j}{b}")
                 for b in range(2)]
                for j in range(2)
            ]

            tiles = {}  # k -> (s_tile, c_tile)

            # bias folded in as a contraction-1 matmul (starts each acc group)
            for jh in range(2):
                for bh in range(2):
                    mm_count[(jh, bh)] += 1
                    nc.tensor.matmul(
                        accs[jh][bh][:],
                        bias_sb[:, jh * P:(jh + 1) * P],
                        ones_sb[:, :],
                        start=True,
                        stop=False,
                    )

            def emit_matmuls(k, w_t, s_t, c_t, split=False):
                # 16 matmuls for this k; rhs slices of the [P, 2048] trig tiles
                for t, trig_t in ((1, s_t), (0, c_t)):
                    for ih in range(2):
                        lhs = w_t[:, (t * 2 + ih) * 256:(t * 2 + ih) * 256 + 256]
                        for jh in range(2):
                            for bh in range(2):
                                cnt = mm_count[(jh, bh)]
                                mm_count[(jh, bh)] = cnt + 1
                                nc.tensor.matmul(
                                    accs[jh][bh][:],
                                    lhs[:, jh * P:(jh + 1) * P],
                                    trig_t[:, ih * 1024 + bh * 512: ih * 1024 + bh * 512 + 512],
                                    start=False,
                                    stop=(cnt == n_mm_per_acc - 1),
                                )

            direct_ct = [0]

            def gen_direct(k, s_t, c_t, halves):
                # range reduction: f = k*x/2pi - round(k*x/2pi)
                sk = float(k / TWO_PI)
                direct_ct[0] += 1
                f_on_dve = direct_ct[0] % 2 == 1
                for lo, hi in halves:
                    u = red_pool.tile([P, F], f32, tag="u")
                    nc.vector.tensor_scalar(u[:, lo:hi], xb[:, lo:hi], sk, None, Alu.mult)
                    v = red_pool.tile([P, F], f32, tag="v")
                    nc.vector.tensor_scalar(v[:, lo:hi], u[:, lo:hi], MAGIC, MAGIC,
                                            Alu.add, Alu.subtract)
                    f_t = red_pool.tile([P, F], f16, tag="f")
                    if f_on_dve:
                        nc.vector.tensor_tensor(f_t[:, lo:hi], u[:, lo:hi], v[:, lo:hi],
                                                Alu.subtract)
                    else:
                        nc.gpsimd.tensor_tensor(f_t[:, lo:hi], u[:, lo:hi], v[:, lo:hi],
                                                Alu.subtract)
                    af = red_pool.tile([P, F], f16, tag="af")
                    nc.scalar.activation(af[:, lo:hi], f_t[:, lo:hi], Act.Abs)
                    nc.scalar.activation(s_t[:, lo:hi], f_t[:, lo:hi], Act.Sin,
                                         bias=0.0, scale=TWO_PI)
                    nc.scalar.activation(c_t[:, lo:hi], af[:, lo:hi], Act.Sin,
                                         bias=pi_half[:], scale=-TWO_PI)

            def gen_double(k, s_t, c_t):
                ps, pc = tiles[DOUBLE[k]]
                sq = red_pool.tile([P, F], f16, tag="sq")
                nc.vector.tensor_tensor(sq[:], ps[:], ps[:], Alu.mult)
                nc.vector.tensor_scalar(c_t[:], sq[:], -2.0, 1.0, Alu.mult, Alu.add)
                sc = red_pool.tile([P, F], f16, tag="sc")
                nc.gpsimd.tensor_tensor(sc[:], ps[:], pc[:], Alu.mult)
                nc.vector.tensor_scalar(s_t[:], sc[:], 2.0, None, Alu.mult)

            def gen_triple(k, s_t, c_t):
                ps, pc = tiles[TRIPLE[k]]
                sq = red_pool.tile([P, F], f16, tag="sq")
                nc.scalar.activation(sq[:], ps[:], Act.Square)
                a = red_pool.tile([P, F], f16, tag="a3")
                nc.vector.tensor_scalar(a[:], sq[:], -4.0, 3.0, Alu.mult, Alu.add)
                nc.vector.tensor_tensor(s_t[:], ps[:], a[:], Alu.mult)
                b_t = red_pool.tile([P, F], f16, tag="a3")
                nc.vector.tensor_scalar(b_t[:], sq[:], -4.0, 1.0, Alu.mult, Alu.add)
                nc.vector.tensor_tensor(c_t[:], pc[:], b_t[:], Alu.mult)

            w_tiles = {}

            def emit_gen(pos, k):
                slot = slot_of[k]
                s_t = trig_pool.tile([P, F], f16, tag=f"ts{slot}")
                c_t = trig_pool.tile([P, F], f16, tag=f"tc{slot}")
                w_t = w_pool.tile([P, 1024], f16, tag="w")
                nc.gpsimd.dma_start(w_t[:], w_dram[k - 1, :, :])
                w_tiles[k] = w_t
                if k in DIRECT:
                    halves = ([(q * 512, (q + 1) * 512) for q in range(4)]
                          if pos == 0 else [(0, F)])
                    gen_direct(k, s_t, c_t, halves)
                elif k in DOUBLE:
                    gen_double(k, s_t, c_t)
                else:
                    gen_triple(k, s_t, c_t)
                tiles[k] = (s_t, c_t)

            DELAY = 3
            for pos, k in enumerate(ORDER):
                emit_gen(pos, k)
                if pos >= DELAY:
                    kk = ORDER[pos - DELAY]
                    emit_matmuls(kk, w_tiles[kk], *tiles[kk])
            for pos in range(len(ORDER) - DELAY, len(ORDER)):
                kk = ORDER[pos]
                emit_matmuls(kk, w_tiles[kk], *tiles[kk])

            # Evacuate PSUM -> SBUF (DVE copy) -> DRAM
            for jh in range(2):
                for bh in range(2):
                    o = out_pool.tile([P, 512], f32, tag="o")
                    nc.vector.tensor_scalar(o[:], accs[jh][bh][:], 1.0, None, Alu.mult)
                    eng = nc.sync if bh == 0 else nc.scalar
                    eng.dma_start(
                        yT_dram[jh * P:(jh + 1) * P, bh * 512:(bh + 1) * 512],
                        o[:],
                    )

    nc.compile()
    _cache["nc"] = nc
    return nc


def _prep_w(fouriercoeffs: np.ndarray) -> np.ndarray:
    # fouriercoeffs: (2, J, I, K) f32 -> (K, 128, 1024) f16 where
    # w[k-1, p, (t*2+ih)*256 + j] = fc[t, j, ih*128+p, k-1]
    fc = np.asarray(fouriercoeffs, dtype=np.float32)  # (2, J, I, K)
    a = fc.transpose(3, 2, 0, 1)                      # (K, I, 2, J)
    a = a.reshape(K, 2, P, 2, J)                      # (k, ih, p, t, j)
    a = a.transpose(0, 2, 3, 1, 4)                    # (k, p, t, ih, j)
    return np.ascontiguousarray(a.reshape(K, P, 4 * J)).astype(np.float16)


def _prep_x(x_shard: np.ndarray) -> np.ndarray:
    # x_shard: (B, I) f32 -> (128, 2048): xb[p, ih*1024 + b] = x[b, ih*128+p]
    xT = np.ascontiguousarray(x_shard.T)              # (I, B)
    return np.ascontiguousarray(
        xT.reshape(2, P, B).transpose(1, 0, 2).reshape(P, 2 * B)
    )


def kernel(x: np.ndarray, fouriercoeffs: np.ndarray, bias: np.ndarray) -> np.ndarray:
    x = np.asarray(x, dtype=np.float32)
    bias = np.asarray(bias, dtype=np.float32)

    nc = _build()
    w_host = _prep_w(fouriercoeffs)
    bias_col = np.ascontiguousarray(bias.reshape(1, J)).astype(np.float16)

    in_maps = []
    for c in range(N_CORES):
        shard = _prep_x(x[c * B:(c + 1) * B])
        in_maps.append({"xb": shard, "w": w_host, "bias": bias_col})

    res = bass_utils.run_bass_kernel_spmd(nc, in_maps, core_ids=list(range(N_CORES)))

    y = np.empty((B_FULL, J), dtype=np.float32)
    for c in range(N_CORES):
        y[c * B:(c + 1) * B] = res.results[c]["yT"].T
    return y


def profile_run(inputs):
    """Run once with tracing enabled; returns BassKernelResults."""
    x = np.asarray(inputs["x"], dtype=np.float32)
    nc = _build()
    w_host = _prep_w(np.asarray(inputs["fouriercoeffs"], dtype=np.float32))
    bias_col = np.ascontiguousarray(
        np.asarray(inputs["bias"], dtype=np.float32).reshape(1, J)
    ).astype(np.float16)
    in_maps = [
        {"xb": _prep_x(x[c * B:(c + 1) * B]), "w": w_host, "bias": bias_col}
        for c in range(N_CORES)
    ]
    return bass_utils.run_bass_kernel_spmd(
        nc, in_maps, core_ids=list(range(N_CORES)), trace=True
    )


# revision 12
# speedup vs baseline: 1.1098x; 1.1098x over previous
"""Trainium2 Bass kernel for KAN Fourier linear layer.

y[b, j] = sum_{i,k} cos(k x[b,i]) W0[j,i,k] + sin(k x[b,i]) W1[j,i,k] + bias[j]

Strategy (8 cores, data-parallel over batch; fp16 matmul; trig generated
on-device with the work spread over DVE, ACT and GpSimd so it all hides
under the PE's ~218us of fp16 matmul):

  - Each core: B=1024 batch rows, all of (i, k, j); W replicated (fp16).
  - Trig tiles are [128, 2048] fp16: partition p = i%128, free dim =
    (i//128)*1024 + b.  PE contracts 128 i-rows per matmul; 64k x 2trig x
    2ih x 2jh x 2bh = 1024 matmuls of [128,128]x[128,512].
  - Per-k generation DAG:
      direct  (26 k): magic-round range reduction (u,v on DVE; f=u-v on
                GpSimd), |f| + sin + cos on ACT (all in one act table).
      double  (27 k): s2=2*s*c, c2=1-2s^2 from parent k/2 (sq,c2,s2 DVE;
                s*c product on GpSimd).
      triple  (11 k): s3=s(3-4s^2), c3=c(1-4s^2) from parent k/3
                (square on ACT; the rest DVE).
    fp16 carry keeps chain error ~2^-11 per hop; worst chain depth 2.
"""

import numpy as np

import concourse.bacc as bacc
import concourse.mybir as mybir
import concourse.tile as tile
from concourse import bass_utils

N_CORES = 8
B_FULL = 8192
B = B_FULL // N_CORES  # 1024 batch rows per core
I = 256
K = 64
J = 256
P = 128
F = 2048  # free dim of trig tiles: (i_half, b)

# Per-k generation plan.
TRIPLE = {3: 1, 9: 3, 15: 5, 21: 7, 33: 11, 39: 13, 45: 15, 51: 17,
          54: 18, 57: 19, 63: 21}
DIRECT = {1, 5, 7, 11, 12, 13, 16, 17, 18, 19, 23, 25, 27, 29, 31, 35,
          37, 41, 43, 47, 49, 53, 55, 59, 60, 61}
DOUBLE = {k: k // 2 for k in range(2, K + 1, 2) if k not in DIRECT and k not in TRIPLE}

# Approximate engine cost (us) per k by class, used to interleave the
# emission order so DVE/ACT/Pool cumulative load stays balanced (the tile
# scheduler follows emission order closely; a run of same-class ks leaves
# two engines idle).
_COST = {
    "direct": (2.44, 6.33, 2.94),   # (DVE, ACT, POOL)
    "double": (2.45, 0.0, 4.16),
    "triple": (3.79, 2.11, 0.0),
}


def _class_of(k):
    return "direct" if k in DIRECT else ("double" if k in DOUBLE else "triple")


def _make_order():
    # directs ordered so chain parents come early
    d_parents = [1, 5, 7, 11, 13, 12, 16, 17, 18, 19, 23, 25, 29, 31]
    head = [1, 5, 2, 7, 3, 4]
    d_rest = [27, 60, 35, 37, 41, 43, 47, 49, 53, 55, 59, 61]
    d_stream = d_parents + d_rest
    emitted = set(head)
    order = list(head)
    load = [0.0, 0.0, 0.0]
    for k in head:
        c = _COST[_class_of(k)]
        load = [load[i] + c[i] for i in range(3)]
    d_stream = [k for k in d_stream if k not in emitted]
    di = 0
    chain_ready = []

    def refresh_ready():
        for k in range(2, K + 1):
            if k in emitted or k in chain_ready or k in DIRECT:
                continue
            par = TRIPLE.get(k) or DOUBLE.get(k)
            if par in emitted:
                chain_ready.append(k)

    while len(order) < K:
        refresh_ready()
        cands = []
        if di < len(d_stream):
            cands.append(("direct", d_stream[di]))
        if chain_ready:
            cands.append((_class_of(chain_ready[0]), chain_ready[0]))
        # pick the candidate minimizing the resulting max engine load
        best = None
        for cls, k in cands:
            c = _COST[cls]
            new = [load[i] + c[i] for i in range(3)]
            key = max(new)
            if best is None or key < best[0]:
                best = (key, cls, k, new)
        _, cls, k, new = best
        load = new
        if cls == "direct" and di < len(d_stream) and d_stream[di] == k:
            di += 1
        else:
            chain_ready.remove(k)
        emitted.add(k)
        order.append(k)
    # keep the tail cheap: swap trailing directs earlier (directs have no
    # parents so moving them earlier is topologically safe; doubles/triples
    # moving later is also safe)
    for pos in range(K - 4, K):
        if order[pos] in DIRECT:
            for q in range(pos - 1, -1, -1):
                if order[q] not in DIRECT and order[q] not in _chain_parents():
                    order[pos], order[q] = order[q], order[pos]
                    break
    return order


def _chain_parents():
    return set(TRIPLE.values()) | set(DOUBLE.values())


ORDER = _make_order()
assert set(ORDER) == set(range(1, K + 1)) and len(ORDER) == K
for k, p_ in {**TRIPLE, **DOUBLE}.items():
    assert ORDER.index(p_) < ORDER.index(k)

TRIG_SLOTS = 14

_cache = {}


def _plan_slots():
    """Assign each k an (s,c) tile slot, register-allocator style."""
    idx = {k: i for i, k in enumerate(ORDER)}
    last_use = {k: min(i + 3, K - 1) for k, i in idx.items()}  # mm DELAY later
    for k in ORDER:
        p_ = TRIPLE.get(k) or DOUBLE.get(k)
        if p_ is not None:
            last_use[p_] = max(last_use[p_], idx[k])
    slot_of = {}
    free = list(range(TRIG_SLOTS))[::-1]
    active = []  # (last_use, k)
    for i, k in enumerate(ORDER):
        active = [(lu, kk) for lu, kk in active if lu >= i]
        used = {slot_of[kk] for _, kk in active}
        free = [s for s in range(TRIG_SLOTS) if s not in used][::-1]
        assert free, f"out of trig slots at k={k}"
        s = free.pop()
        slot_of[k] = s
        active.append((last_use[k], k))
    return slot_of


def _build():
    if "nc" in _cache:
        return _cache["nc"]

    f32 = mybir.dt.float32
    f16 = mybir.dt.float16
    u32 = mybir.dt.uint32
    nc = bacc.Bacc("TRN2", target_bir_lowering=False, debug=False, num_devices=N_CORES)

    xb_dram = nc.dram_tensor("xb", (P, F), f32, kind="ExternalInput")
    w_dram = nc.dram_tensor("w", (K, P, 1024), f16, kind="ExternalInput")
    bias_dram = nc.dram_tensor("bias", (1, J), f16, kind="ExternalInput")
    yT_dram = nc.dram_tensor("yT", (J, B), f32, kind="ExternalOutput")

    TWO_PI = float(2.0 * np.pi)
    PI = float(np.pi)
    MAGIC = float(1.5 * 2.0**23)
    Alu = mybir.AluOpType
    Act = mybir.ActivationFunctionType

    slot_of = _plan_slots()
    n_mm_per_acc = K * 4 + 1  # trig x ih per k, plus the bias matmul
    mm_count = {(jh, bh): 0 for jh in range(2) for bh in range(2)}

    with tile.TileContext(nc) as tc:
        with (
            tc.tile_pool(name="const", bufs=1) as const_pool,
            tc.tile_pool(name="wload", bufs=5) as w_pool,
            tc.tile_pool(name="red", bufs=2) as red_pool,
            tc.tile_pool(name="trig", bufs=1) as trig_pool,
            tc.tile_pool(name="psum", bufs=1, space="PSUM") as psum_pool,
            tc.tile_pool(name="out", bufs=2) as out_pool,
        ):
            xb = const_pool.tile([P, F], f32, tag="xb")
            # split the x DMA so the first seed can start sooner
            for q in range(4):
                nc.sync.dma_start(xb[:, q * 512:(q + 1) * 512],
                                  xb_dram[:, q * 512:(q + 1) * 512])

            bias_sb = const_pool.tile([1, J], f16, tag="bias")
            nc.sync.dma_start(bias_sb[:], bias_dram[:, :])
            ones_sb = const_pool.tile([1, 512], f16, tag="ones")
            nc.vector.memset(ones_sb[:], 1.0)
            pi_half = const_pool.tile([P, 1], f32, tag="pi_half")
            nc.vector.memset(pi_half[:], PI / 2)

            accs = [
                [psum_pool.tile([P, 512], f32, tag=f"acc{j}{b}", name=f"acc{j}{b}")
                 for b in range(2)]
                for j in range(2)
            ]

            tiles = {}  # k -> (s_tile, c_tile)

            # bias folded in as a contraction-1 matmul (starts each acc group)
            for jh in range(2):
                for bh in range(2):
                    mm_count[(jh, bh)] += 1
                    nc.tensor.matmul(
                        accs[jh][bh][:],
                        bias_sb[:, jh * P:(jh + 1) * P],
                        ones_sb[:, :],
                        start=True,
                        stop=False,
                    )

            def emit_matmuls(k, w_t, s_t, c_t, split=False):
                # 16 matmuls for this k; rhs slices of the [P, 2048] trig tiles
                for t, trig_t in ((1, s_t), (0, c_t)):
                    for ih in range(2):
                        lhs = w_t[:, (t * 2 + ih) * 256:(t * 2 + ih) * 256 + 256]
                        for jh in range(2):
                            for bh in range(2):
                                cnt = mm_count[(jh, bh)]
                                mm_count[(jh, bh)] = cnt + 1
                                nc.tensor.matmul(
                                    accs[jh][bh][:],
                                    lhs[:, jh * P:(jh + 1) * P],
                                    trig_t[:, ih * 1024 + bh * 512: ih * 1024 + bh * 512 + 512],
                                    start=False,
                                    stop=(cnt == n_mm_per_acc - 1),
                                )

            direct_ct = [0]

            def gen_direct(k, s_t, c_t, halves):
                # range reduction: f = k*x/2pi - round(k*x/2pi)
                sk = float(k / TWO_PI)
                direct_ct[0] += 1
                f_on_dve = direct_ct[0] % 2 == 1
                for lo, hi in halves:
                    u = red_pool.tile([P, F], f32, tag="u")
                    nc.vector.tensor_scalar(u[:, lo:hi], xb[:, lo:hi], sk, None, Alu.mult)
                    v = red_pool.tile([P, F], f32, tag="v")
                    nc.vector.tensor_scalar(v[:, lo:hi], u[:, lo:hi], MAGIC, MAGIC,
                                            Alu.add, Alu.subtract)
                    f_t = red_pool.tile([P, F], f16, tag="f")
                    if f_on_dve:
                        nc.vector.tensor_tensor(f_t[:, lo:hi], u[:, lo:hi], v[:, lo:hi],
                                                Alu.subtract)
                    else:
                        nc.gpsimd.tensor_tensor(f_t[:, lo:hi], u[:, lo:hi], v[:, lo:hi],
                                                Alu.subtract)
                    af = red_pool.tile([P, F], f16, tag="af")
                    nc.scalar.activation(af[:, lo:hi], f_t[:, lo:hi], Act.Abs)
                    nc.scalar.activation(s_t[:, lo:hi], f_t[:, lo:hi], Act.Sin,
                                         bias=0.0, scale=TWO_PI)
                    nc.scalar.activation(c_t[:, lo:hi], af[:, lo:hi], Act.Sin,
                                         bias=pi_half[:], scale=-TWO_PI)

            def gen_double(k, s_t, c_t):
                ps, pc = tiles[DOUBLE[k]]
                sq = red_pool.tile([P, F], f16, tag="sq")
                nc.vector.tensor_tensor(sq[:], ps[:], ps[:], Alu.mult)
                nc.vector.tensor_scalar(c_t[:], sq[:], -2.0, 1.0, Alu.mult, Alu.add)
                sc = red_pool.tile([P, F], f16, tag="sc")
                nc.gpsimd.tensor_tensor(sc[:], ps[:], pc[:], Alu.mult)
                nc.vector.tensor_scalar(s_t[:], sc[:], 2.0, None, Alu.mult)

            def gen_triple(k, s_t, c_t):
                ps, pc = tiles[TRIPLE[k]]
                sq = red_pool.tile([P, F], f16, tag="sq")
                nc.scalar.activation(sq[:], ps[:], Act.Square)
                a = red_pool.tile([P, F], f16, tag="a3")
                nc.vector.tensor_scalar(a[:], sq[:], -4.0, 3.0, Alu.mult, Alu.add)
                nc.vector.tensor_tensor(s_t[:], ps[:], a[:], Alu.mult)
                b_t = red_pool.tile([P, F], f16, tag="a3")
                nc.vector.tensor_scalar(b_t[:], sq[:], -4.0, 1.0, Alu.mult, Alu.add)
                nc.vector.tensor_tensor(c_t[:], pc[:], b_t[:], Alu.mult)

            w_tiles = {}

            def emit_gen(pos, k):
                slot = slot_of[k]
                s_t = trig_pool.tile([P, F], f16, tag=f"ts{slot}")
                c_t = trig_pool.tile([P, F], f16, tag=f"tc{slot}")
                w_t = w_pool.tile([P, 1024], f16, tag="w")
                nc.scalar.dma_start(w_t[:], w_dram[k - 1, :, :])
                w_tiles[k] = w_t
                if k in DIRECT:
                    halves = ([(q * 512, (q + 1) * 512) for q in range(4)]
                          if pos == 0 else [(0, F)])
                    gen_direct(k, s_t, c_t, halves)
                elif k in DOUBLE:
                    gen_double(k, s_t, c_t)
                else:
                    gen_triple(k, s_t, c_t)
                tiles[k] = (s_t, c_t)

            DELAY = 3
            for pos, k in enumerate(ORDER):
                emit_gen(pos, k)
                if pos >= DELAY:
                    kk = ORDER[pos - DELAY]
                    emit_matmuls(kk, w_tiles[kk], *tiles[kk])
            for pos in range(len(ORDER) - DELAY, len(ORDER)):
                kk = ORDER[pos]
                emit_matmuls(kk, w_tiles[kk], *tiles[kk])

            # Evacuate PSUM -> SBUF (DVE copy) -> DRAM
            for jh in range(2):
                for bh in range(2):
                    o = out_pool.tile([P, 512], f32, tag="o")
                    nc.vector.tensor_scalar(o[:], accs[jh][bh][:], 1.0, None, Alu.mult)
                    eng = nc.sync if bh == 0 else nc.scalar
                    eng.dma_start(
                        yT_dram[jh * P:(jh + 1) * P, bh * 512:(bh + 1) * 512],
                        o[:],
                    )

    nc.compile()
    _cache["nc"] = nc
    return nc


def _prep_w(fouriercoeffs: np.ndarray) -> np.ndarray:
    # fouriercoeffs: (2, J, I, K) f32 -> (K, 128, 1024) f16 where
    # w[k-1, p, (t*2+ih)*256 + j] = fc[t, j, ih*128+p, k-1]
    fc = np.asarray(fouriercoeffs, dtype=np.float32)  # (2, J, I, K)
    a = fc.transpose(3, 2, 0, 1)                      # (K, I, 2, J)
    a = a.reshape(K, 2, P, 2, J)                      # (k, ih, p, t, j)
    a = a.transpose(0, 2, 3, 1, 4)                    # (k, p, t, ih, j)
    return np.ascontiguousarray(a.reshape(K, P, 4 * J)).astype(np.float16)


def _prep_x(x_shard: np.ndarray) -> np.ndarray:
    # x_shard: (B, I) f32 -> (128, 2048): xb[p, ih*1024 + b] = x[b, ih*128+p]
    xT = np.ascontiguousarray(x_shard.T)              # (I, B)
    return np.ascontiguousarray(
        xT.reshape(2, P, B).transpose(1, 0, 2).reshape(P, 2 * B)
    )


def kernel(x: np.ndarray, fouriercoeffs: np.ndarray, bias: np.ndarray) -> np.ndarray:
    x = np.asarray(x, dtype=np.float32)
    bias = np.asarray(bias, dtype=np.float32)

    nc = _build()
    w_host = _prep_w(fouriercoeffs)
    bias_col = np.ascontiguousarray(bias.reshape(1, J)).astype(np.float16)

    in_maps = []
    for c in range(N_CORES):
        shard = _prep_x(x[c * B:(c + 1) * B])
        in_maps.append({"xb": shard, "w": w_host, "bias": bias_col})

    res = bass_utils.run_bass_kernel_spmd(nc, in_maps, core_ids=list(range(N_CORES)))

    y = np.empty((B_FULL, J), dtype=np.float32)
    for c in range(N_CORES):
        y[c * B:(c + 1) * B] = res.results[c]["yT"].T
    return y


def profile_run(inputs):
    """Run once with tracing enabled; returns BassKernelResults."""
    x = np.asarray(inputs["x"], dtype=np.float32)
    nc = _build()
    w_host = _prep_w(np.asarray(inputs["fouriercoeffs"], dtype=np.float32))
    bias_col = np.ascontiguousarray(
        np.asarray(inputs["bias"], dtype=np.float32).reshape(1, J)
    ).astype(np.float16)
    in_maps = [
        {"xb": _prep_x(x[c * B:(c + 1) * B]), "w": w_host, "bias": bias_col}
        for c in range(N_CORES)
    ]
    return bass_utils.run_bass_kernel_spmd(
        nc, in_maps, core_ids=list(range(N_CORES)), trace=True
    )
